# revision 1
# baseline (speedup 1.0000x reference)
"""Correlation layer (FlowNet-style) on 8 Trainium2 NeuronCores.

Strategy (data-parallel over batch, one batch element per core):
  out[d,h,w] = sum_c x1[c,h,w] * x2p[c, h+di+4, w+dj+4] / sqrt(C),
  di,dj in [-4,4], 80 displacements (81 minus center).

Per core, banded-Gram matmuls with displacement packing:
  - lhsT = x1 rows [24g-8+rho'' : +32) x 4 w-cols  -> M = 32*4 = 128
    (m = rho''*4 + ww, rho'' = rr - di + 4)
  - rhs  = x2p rows [24g : 24g+24) x 12 padded-w cols -> N = 24*12 = 288
    (n = rr*12 + u, u = ww + dj + 4)
  - psum[m, n] is useful iff rho'' = rr - di + 4 and u - ww in [0, 9).
    For fixed rr all useful elements live in partitions [4rr, 4rr+36),
    so a pure-partition-step DMA can ship a 75%-dense window per rr.

Pipeline: DMA x2p slab (24 rows) -> 288-col f32r/bf16/f32 matmuls ->
DVE/ACT copy PSUM->SBUF staging (relayout to (rr, wb, u)) -> per-rr
window DMA (576B contiguous runs) -> DRAM; host decodes windows into
the [80, H, W] layout with pure slicing.
"""

import math
import numpy as np
from contextlib import ExitStack

B, C, H, W = 8, 128, 128, 192
MD = 4
NDISP = 81

R = 24          # x2p rows per group
NG = 6          # row groups (covers 144 padded rows)
WW = 4          # output w-cols per block
NWB = W // WW   # 48 blocks
CHUNK = 12      # blocks per staging chunk
NCH = NWB // CHUNK  # 4
HP = 152        # x1pad rows: 8 zero + 128 + 16 zero
X1B = NWB * 32 * WW        # per-group x1 block slab: 6144 elems/partition
X2R, X2C = NG * R, W + 8   # 144 x 200
UB = WW + 8     # 12 rhs cols per block
NMM = R * UB    # 288 matmul free size
ROWSZ = R * CHUNK * UB     # staging free size 3456
WIN = 36                   # band window partitions per rr
OUTSZ = NG * NCH * R * WIN * (CHUNK * UB)  # per-core band elements

MM_DTYPE = "float32"      # "float32" | "float32r" | "bfloat16"
EVAC_PATTERN = "vvs"       # per-copy engine cycle: v=vector, s=scalar

_CACHE = {}


def _build(mm_dtype, evac_pattern):
    import concourse.bass as bass
    import concourse.tile as tile
    from concourse import bacc, mybir

    in_dt = mybir.dt.bfloat16 if mm_dtype == "bfloat16" else mybir.dt.float32
    f32 = mybir.dt.float32

    nc = bacc.Bacc("TRN2", target_bir_lowering=False, debug=False, num_devices=8)
    x1d = nc.dram_tensor("x1p", [128, NG * X1B], in_dt, kind="ExternalInput")
    x2d = nc.dram_tensor("x2p", [128, X2R * X2C], in_dt, kind="ExternalInput")
    outd = nc.dram_tensor("band", [OUTSZ], f32, kind="ExternalOutput")

    with tile.TileContext(nc) as tc, ExitStack() as ctx:
        x1pool = ctx.enter_context(tc.tile_pool(name="x1", bufs=2))
        x2pool = ctx.enter_context(tc.tile_pool(name="x2", bufs=2))
        pspool = ctx.enter_context(tc.tile_pool(name="ps", bufs=4, space="PSUM"))
        stpool = ctx.enter_context(tc.tile_pool(name="st", bufs=3))

        ev = 0
        for g in range(NG):
            x1t = x1pool.tile([128, X1B], in_dt, tag="x1")
            nc.sync.dma_start(x1t[:], x1d.ap()[:, g * X1B:(g + 1) * X1B])
            x2t = x2pool.tile([128, R * X2C], in_dt, tag="x2")
            nc.sync.dma_start(x2t[:], x2d.ap()[:, g * R * X2C:(g + 1) * R * X2C])
            x2v = x2t[:].rearrange("p (r u) -> p r u", r=R)
            for chn in range(NCH):
                stt = stpool.tile([128, ROWSZ], f32, tag="st")
                stv = stt[:].rearrange("p (r b u) -> p r b u", r=R, b=CHUNK)
                for wp in range(CHUNK // 2):
                    pst = pspool.tile([128, 2, 512], f32, tag="ps")
                    for k in range(2):
                        wb = chn * CHUNK + wp * 2 + k
                        lhsT = x1t[:, wb * 128:(wb + 1) * 128]
                        rhs = x2v[:, :, wb * WW:wb * WW + UB]
                        if mm_dtype == "float32r":
                            lhsT = lhsT.bitcast(mybir.dt.float32r)
                            rhs = rhs.bitcast(mybir.dt.float32r)
                        nc.tensor.matmul(pst[:, k, 0:NMM], lhsT, rhs,
                                         start=True, stop=True)
                    # evac pair -> staging (rr, wb_local, u), (k, rr, u)->(rr, k, u)
                    src = pst[:, :, 0:NMM].rearrange(
                        "p a (r u) -> p r a u", r=R).copy()
                    dst = stv[:, :, wp * 2:wp * 2 + 2, :]
                    if evac_pattern[ev % len(evac_pattern)] == "v":
                        nc.vector.tensor_copy(dst, src)
                    else:
                        nc.scalar.copy(dst, src)
                    ev += 1
                # band window DMAs: one per rr
                for rr in range(R):
                    src = bass.AP(stt[:].tensor,
                                  (4 * rr) * ROWSZ + rr * (CHUNK * UB),
                                  [[ROWSZ, WIN], [1, CHUNK * UB]])
                    dsto = ((g * NCH + chn) * R + rr) * WIN * (CHUNK * UB)
                    dst = bass.AP(outd.ap().tensor, dsto,
                                  [[CHUNK * UB, WIN], [1, CHUNK * UB]])
                    nc.sync.dma_start(dst, src)

    nc.compile()
    return nc


def _get_nc():
    key = (MM_DTYPE, EVAC_PATTERN)
    if key not in _CACHE:
        _CACHE[key] = _build(*key)
    return _CACHE[key]


def _prep_inputs(x1, x2):
    import ml_dtypes
    np_dt = ml_dtypes.bfloat16 if MM_DTYPE == "bfloat16" else np.float32
    in_maps = []
    for b in range(B):
        x1p = np.zeros((128, HP, NWB, WW), np_dt)
        x1p.reshape(128, HP, W)[:, 8:8 + H, :] = np.asarray(x1[b], np.float32)
        win = np.stack([x1p[:, R * g:R * g + 32] for g in range(NG)], axis=1)
        x1b = win.transpose(0, 1, 3, 2, 4).reshape(128, NG * X1B)
        x2p = np.zeros((128, X2R, X2C), np_dt)
        x2p[:, 4:4 + H, 4:4 + W] = np.asarray(x2[b], np.float32)
        in_maps.append({"x1p": np.ascontiguousarray(x1b),
                        "x2p": x2p.reshape(128, X2R * X2C)})
    return in_maps


def _decode(band, out81):
    """band: per-core [OUTSZ] f32 -> out81 [81, H, W] (scaled later)."""
    arr = band.reshape(NG, NCH, R, 9, 4, CHUNK, UB)  # (g,c,rr,t,ww,wb,u)
    for ww in range(WW):
        sub = arr[:, :, :, :, ww, :, ww:ww + 9]       # (g,c,rr,t,wb,dj)
        tmat = sub.transpose(3, 5, 0, 2, 1, 4).reshape(9, 9, NG * R, NCH * CHUNK)
        for t in range(9):
            di_idx = 8 - t                            # di = 4 - t
            r2lo = di_idx                             # r2 = h + di + 4
            out81[di_idx * 9:di_idx * 9 + 9, :, ww::WW] = \
                tmat[t, :, r2lo:r2lo + H, :]
    return out81


def kernel(x1, x2):
    from concourse.bass_utils import run_bass_kernel_spmd

    x1 = np.asarray(x1, np.float32)
    x2 = np.asarray(x2, np.float32)
    nc = _get_nc()
    in_maps = _prep_inputs(x1, x2)
    res = run_bass_kernel_spmd(nc, in_maps, core_ids=list(range(8)))

    inv_sqrt_c = np.float32(1.0 / math.sqrt(C))
    out = np.empty((B, NDISP - 1, H, W), np.float32)
    out81 = np.empty((NDISP, H, W), np.float32)
    for b in range(B):
        _decode(res.results[b]["band"], out81)
        out[b] = np.delete(out81, 40, axis=0) * inv_sqrt_c
    return out



# revision 2
# speedup vs baseline: 85.1296x; 85.1296x over previous
"""Correlation layer (FlowNet-style) on 8 Trainium2 NeuronCores — v3.

Data-parallel over batch (one element per core). Per core:
  out[d,h,w] = sum_c x1[c,h,w] * x2p[c, h+di+4, w+dj+4] / sqrt(C)

Design (walrus-verifier-legal):
  - fp16 inputs + fp16 band output (half the HBM traffic); fp16 matmul
    runs at 1 cycle/row (4x over fp32).
  - x1 host-packed per group as (wb, rho32, ww4) so each (g, wb) weight
    block is one contiguous 128-elem run (weights AP must be 1-D).
  - x2 SBUF-resident, shipped unpadded; h-pad rows memset on device;
    w-pad eliminated by clamping the rhs column window at the two edge
    wb blocks + host-side zeroing of w-out-of-bounds outputs (the
    reference zero-pads, so those outputs are exactly 0).
  - Banded-Gram matmuls: lhsT = x1 block [C, (rho32, ww4)=128],
    rhs = x2 rows [C, (r24, u12)=288] -> psum[(rho,ww), (r,u)].
  - Evac psum->staging fp16 with (r, wb, u) relayout on DVE/ACT
    (GPSIMD cannot read PSUM).
  - Band DMA per rr-QUAD: 4 consecutive rr share one 48-partition
    window [4rr0, 4rr0+48) x 4*576 free elems — clean partition steps,
    36 DMAs/core, 75% useful density.

Band layout per core: (g, q6, w48, j4, wb48, u12) fp16 where for
rr = 4q + j: w = 4*(j + Delta) + ww, di = 4 - Delta, u = ww + dj + 4.
Host decodes with pure slicing + boundary zeroing.
"""

import math
import numpy as np
from contextlib import ExitStack

B, C, H, W = 8, 128, 128, 192
MD = 4
NDISP = 81

R = 24              # x2p rows per group
NG = 6              # row groups (cover 144 padded x2 rows)
WW = 4              # output w-cols per block
NWB = W // WW       # 48 blocks (all in one chunk)
UB = WW + 8         # 12 rhs cols per block
NMM = R * UB        # 288 matmul free size
ROWSZ = R * NWB * UB        # staging free size 13824
NQ = R // 4                 # 6 rr-quads per group
QWIN = 48                   # band window partitions per quad
QFREE = 4 * NWB * UB        # 2304 free elems per quad window
BANDSZ = NG * NQ * QWIN * QFREE

X1ROWS = 152        # 8 zero + 128 + 16 zero (host-side pad)
X2ROWS = 144        # 4 zero + 128 + 12 zero
X1B = NWB * 32 * WW  # per-group packed x1 slab: 6144 elems/partition

EVAC_PATTERN = "vs"  # per-copy engine cycle: v=DVE, s=ACT
X2_ENG = "p"         # engine issuing x2 input DMAs (p=Pool SWDGE)
X1_ENG = "y"         # engine issuing x1 input DMAs (y=SP)
BAND_ENG = "yp"      # engines issuing band DMAs (cycled per quad)
PS_BANKS = 2         # psum banks per tile (matmuls per evac copy)
MS_ENG = "v"         # engine for x2 h-pad memsets
ST_BUFS = 2          # staging pool depth

_CACHE = {}


def _build(cfg=None):
    import concourse.bass as bass
    import concourse.tile as tile
    from concourse import bacc, mybir

    f16 = mybir.dt.float16
    f32 = mybir.dt.float32

    nc = bacc.Bacc("TRN2", target_bir_lowering=False, debug=False, num_devices=8)
    x1d = nc.dram_tensor("x1h", [128, NG * X1B], f16, kind="ExternalInput")
    x2d = nc.dram_tensor("x2h", [128, H * W], f16, kind="ExternalInput")
    outd = nc.dram_tensor("band", [BANDSZ], f16, kind="ExternalOutput")

    with tile.TileContext(nc) as tc, ExitStack() as ctx:
        x1pool = ctx.enter_context(tc.tile_pool(name="x1", bufs=1))
        x2pool = ctx.enter_context(tc.tile_pool(name="x2", bufs=1))
        pspool = ctx.enter_context(
            tc.tile_pool(name="ps", bufs=8 // PS_BANKS, space="PSUM"))
        stpool = ctx.enter_context(tc.tile_pool(name="st", bufs=ST_BUFS))

        x1t = x1pool.tile([128, NG * X1B], f16, tag="x1")
        x2t = x2pool.tile([128, X2ROWS * W], f16, tag="x2")
        x2v = x2t[:].rearrange("p (r w) -> p r w", r=X2ROWS)

        eng = {"v": nc.vector, "s": nc.scalar, "p": nc.gpsimd, "y": nc.sync}

        # zero x2 h-pad rows (once)
        eng[MS_ENG].memset(x2v[:, 0:4, :], 0)
        eng[MS_ENG].memset(x2v[:, 132:144, :], 0)

        # x2 interior per group (tile rows 4..132 <- true rows 0..128)
        for g in range(NG):
            a = 4 if g == 0 else 24 * g
            b = min(132, 24 * g + 24)
            eng[X2_ENG].dma_start(
                x2v[:, a:b, :],
                x2d.ap()[:, (a - 4) * W:(b - 4) * W])

        ev = 0
        bq = 0
        for g in range(NG):
            # last group: x2p rows 136..144 are never decoded — compute
            # only rr 0..16 (quads 0..3)
            rg = 16 if g == NG - 1 else R
            nmm_g = rg * UB
            # x1 slab for this group (SP queue, interleaved with band DMAs)
            eng[X1_ENG].dma_start(
                x1t[:, g * X1B:(g + 1) * X1B],
                x1d.ap()[:, g * X1B:(g + 1) * X1B])

            stt = stpool.tile([128, ROWSZ], f16, tag="st")
            stv = stt[:].rearrange("p (r b u) -> p r b u", r=R, b=NWB)
            nb = PS_BANKS
            for q in range(NWB // nb):
                pst = pspool.tile([128, nb, 512], f32, tag="ps")
                for k in range(nb):
                    wb = q * nb + k
                    c0 = min(max(4 * wb - 4, 0), W - UB)
                    lhsT = x1t[:, g * X1B + wb * 128:g * X1B + (wb + 1) * 128]
                    rhs = x2v[:, 24 * g:24 * g + rg, c0:c0 + UB]
                    nc.tensor.matmul(pst[:, k, 0:nmm_g], lhsT, rhs,
                                     start=True, stop=True)

                has_lo = q == 0                       # k=0 is wb 0
                has_hi = q == NWB // nb - 1           # last k is wb 47
                ch = EVAC_PATTERN[ev % len(EVAC_PATTERN)]
                cp = {"v": nc.vector.tensor_copy, "s": nc.scalar.copy}[ch]
                ev += 1
                b0 = q * nb
                if has_lo:
                    # wb=0 clamped: psum u' = w2, dst u = u'+4
                    p0 = pst[:, 0, 0:nmm_g].rearrange("p (r u) -> p r u", r=rg)
                    cp(stv[:, 0:rg, 0, 4:12], p0[:, :, 0:8].copy())
                    if nb > 1:
                        src = pst[:, 1:nb, 0:nmm_g].rearrange(
                            "p a (r u) -> p r a u", r=rg).copy()
                        cp(stv[:, 0:rg, 1:nb, :], src)
                elif has_hi:
                    # wb=47 clamped: psum u' = w2-180, dst u = u'-4
                    p3 = pst[:, nb - 1, 0:nmm_g].rearrange(
                        "p (r u) -> p r u", r=rg)
                    cp(stv[:, 0:rg, NWB - 1, 0:8], p3[:, :, 4:12].copy())
                    if nb > 1:
                        src = pst[:, 0:nb - 1, 0:nmm_g].rearrange(
                            "p a (r u) -> p r a u", r=rg).copy()
                        cp(stv[:, 0:rg, b0:b0 + nb - 1, :], src)
                else:
                    src = pst[:, :, 0:nmm_g].rearrange(
                        "p a (r u) -> p r a u", r=rg).copy()
                    cp(stv[:, 0:rg, b0:b0 + nb, :], src)

            # band DMAs: one per rr-quad, clean partition steps
            for q in range(rg // 4):
                rr0 = 4 * q
                src = bass.AP(stt[:].tensor,
                              (4 * rr0) * ROWSZ + rr0 * (NWB * UB),
                              [[ROWSZ, QWIN], [1, QFREE]])
                dsto = (g * NQ + q) * QWIN * QFREE
                dst = bass.AP(outd.ap().tensor, dsto,
                              [[QFREE, QWIN], [1, QFREE]])
                be = BAND_ENG[bq % len(BAND_ENG)]
                bq += 1
                eng[be].dma_start(dst, src)

    nc.compile()
    return nc


def _get_nc():
    key = (EVAC_PATTERN, X2_ENG, X1_ENG, BAND_ENG, PS_BANKS, MS_ENG, ST_BUFS)
    if key not in _CACHE:
        _CACHE[key] = _build()
    return _CACHE[key]


def _prep_inputs(x1, x2):
    x1 = np.asarray(x1, np.float32)
    x2 = np.asarray(x2, np.float32)
    in_maps = []
    for b in range(B):
        x1p = np.zeros((128, X1ROWS, W), np.float16)
        x1p[:, 8:8 + H, :] = x1[b]
        s0, s1, s2 = x1p.strides
        win = np.lib.stride_tricks.as_strided(
            x1p, shape=(128, NG, 32, NWB, WW),
            strides=(s0, 24 * s1, s1, WW * s2, s2))
        x1b = win.transpose(0, 1, 3, 2, 4).reshape(128, NG * X1B)
        in_maps.append({
            "x1h": np.ascontiguousarray(x1b),
            "x2h": x2[b].reshape(128, H * W).astype(np.float16)})
    return in_maps


def _decode(band, out81):
    """band: per-core [BANDSZ] f16/f32 -> out81 [81, H, W] f32 (unscaled)."""
    # (g*q, w//4=tprime, ww, j, wb, u)
    arr = band.reshape(NG * NQ, 12, WW, 4, NWB, UB)
    tm = np.empty((9, 9, X2ROWS, W), np.float32)  # (Delta, dj, rp, w')
    for ww in range(WW):
        for j in range(4):
            sub = arr[:, j:j + 9, ww, j, :, ww:ww + 9]  # (gq, Delta, wb, dj)
            tm[:, :, j::4, ww::WW] = sub.transpose(1, 3, 0, 2)
    for t in range(9):
        di_idx = 8 - t                            # di = 4 - t
        r2lo = di_idx                             # rp = h + di + 4
        out81[di_idx * 9:di_idx * 9 + 9] = tm[t, :, r2lo:r2lo + H, :]
    # w-boundary: reference zero-pads x2, so outputs with w+dj out of
    # bounds are exactly 0 (the device leaves garbage there).
    v = out81.reshape(9, 9, H, W)
    for dj in range(-MD, MD + 1):
        if dj < 0:
            v[:, dj + 4, :, 0:-dj] = 0
        elif dj > 0:
            v[:, dj + 4, :, W - dj:] = 0
    return out81


def kernel(x1, x2):
    from concourse.bass_utils import run_bass_kernel_spmd

    nc = _get_nc()
    in_maps = _prep_inputs(x1, x2)
    res = run_bass_kernel_spmd(nc, in_maps, core_ids=list(range(8)))

    inv_sqrt_c = np.float32(1.0 / math.sqrt(C))
    out = np.empty((B, NDISP - 1, H, W), np.float32)
    out81 = np.empty((NDISP, H, W), np.float32)
    for b in range(B):
        _decode(res.results[b]["band"].astype(np.float32), out81)
        out[b] = np.delete(out81, 40, axis=0) * inv_sqrt_c
    return out


# revision 5
# speedup vs baseline: 722.7069x; 8.4895x over previous
"""Correlation layer (FlowNet-style) on 8 Trainium2 NeuronCores — v3.

Data-parallel over batch (one element per core). Per core:
  out[d,h,w] = sum_c x1[c,h,w] * x2p[c, h+di+4, w+dj+4] / sqrt(C)

Design (walrus-verifier-legal):
  - fp16 inputs + fp16 band output (half the HBM traffic); fp16 matmul
    runs at 1 cycle/row (4x over fp32).
  - x1 host-packed per group as (wb, rho32, ww4) so each (g, wb) weight
    block is one contiguous 128-elem run (weights AP must be 1-D).
  - x2 SBUF-resident, shipped unpadded; h-pad rows memset on device;
    w-pad eliminated by clamping the rhs column window at the two edge
    wb blocks + host-side zeroing of w-out-of-bounds outputs (the
    reference zero-pads, so those outputs are exactly 0).
  - Banded-Gram matmuls: lhsT = x1 block [C, (rho32, ww4)=128],
    rhs = x2 rows [C, (r24, u12)=288] -> psum[(rho,ww), (r,u)].
  - Evac psum->staging fp16 with (r, wb, u) relayout on DVE/ACT
    (GPSIMD cannot read PSUM).
  - Band DMA per rr-QUAD: 4 consecutive rr share one 48-partition
    window [4rr0, 4rr0+48) x 4*576 free elems — clean partition steps,
    36 DMAs/core, 75% useful density.

Band layout per core: (g, q6, w48, j4, wb48, u12) fp16 where for
rr = 4q + j: w = 4*(j + Delta) + ww, di = 4 - Delta, u = ww + dj + 4.
Host decodes with pure slicing + boundary zeroing.
"""

import math
import numpy as np
from contextlib import ExitStack

B, C, H, W = 8, 128, 128, 192
MD = 4
NDISP = 81

R = 24              # x2p rows per group
NG = 6              # row groups (cover 144 padded x2 rows)
WW = 4              # output w-cols per block
NWB = W // WW       # 48 blocks (all in one chunk)
UB = WW + 8         # 12 rhs cols per block
NMM = R * UB        # 288 matmul free size
ROWSZ = R * NWB * UB        # staging free size 13824
NQ = R // 4                 # 6 rr-quads per group
QWIN = 48                   # band window partitions per quad
QFREE = 4 * NWB * UB        # 2304 free elems per quad window
BANDSZ = NG * NQ * QWIN * QFREE

X1ROWS = 152        # 8 zero + 128 + 16 zero (host-side pad)
X2ROWS = 144        # 4 zero + 128 + 12 zero
X1B = NWB * 32 * WW  # per-group packed x1 slab: 6144 elems/partition

EVAC_PATTERN = "vs"  # per-copy engine cycle: v=DVE, s=ACT
X2_ENG = "p"         # engine issuing x2 input DMAs (p=Pool SWDGE)
X1_ENG = "y"         # engine issuing x1 input DMAs (y=SP)
BAND_ENG = "yp"      # engines issuing band DMAs (cycled per quad)
PS_BANKS = 2         # psum banks per tile (matmuls per evac copy)
MS_ENG = "v"         # engine for x2 h-pad memsets
ST_BUFS = 2          # staging pool depth
LOOP_N = 1           # >1: wrap the whole kernel in a device For_i loop
                     # (timing-only builds; kernel() always uses 1)

_CACHE = {}


def _build(cfg=None):
    import concourse.bass as bass
    import concourse.tile as tile
    from concourse import bacc, mybir

    f16 = mybir.dt.float16
    f32 = mybir.dt.float32

    nc = bacc.Bacc("TRN2", target_bir_lowering=False, debug=False, num_devices=8)
    x1d = nc.dram_tensor("x1h", [128, NG * X1B], f16, kind="ExternalInput")
    x2d = nc.dram_tensor("x2h", [128, H * W], f16, kind="ExternalInput")
    outd = nc.dram_tensor("band", [BANDSZ], f16, kind="ExternalOutput")

    with tile.TileContext(nc) as tc, ExitStack() as ctx:
        x1pool = ctx.enter_context(tc.tile_pool(name="x1", bufs=1))
        x2pool = ctx.enter_context(tc.tile_pool(name="x2", bufs=1))
        pspool = ctx.enter_context(
            tc.tile_pool(name="ps", bufs=8 // PS_BANKS, space="PSUM"))
        stpool = ctx.enter_context(tc.tile_pool(name="st", bufs=ST_BUFS))

        x1t = x1pool.tile([128, NG * X1B], f16, tag="x1")
        x2t = x2pool.tile([128, X2ROWS * W], f16, tag="x2")
        x2v = x2t[:].rearrange("p (r w) -> p r w", r=X2ROWS)

        eng = {"v": nc.vector, "s": nc.scalar, "p": nc.gpsimd, "y": nc.sync}

        def body():
            # zero x2 h-pad rows
            eng[MS_ENG].memset(x2v[:, 0:4, :], 0)
            eng[MS_ENG].memset(x2v[:, 132:144, :], 0)

            # x2 interior per group (tile rows 4..132 <- true rows 0..128)
            for g in range(NG):
                a = 4 if g == 0 else 24 * g
                b = min(132, 24 * g + 24)
                eng[X2_ENG].dma_start(
                    x2v[:, a:b, :],
                    x2d.ap()[:, (a - 4) * W:(b - 4) * W])

            emit_main()

        def emit_main():
          ev = 0
          bq = 0
          for g in range(NG):
            # last group: x2p rows 136..144 are never decoded — compute
            # only rr 0..16 (quads 0..3)
            rg = 16 if g == NG - 1 else R
            nmm_g = rg * UB
            # x1 slab for this group (SP queue, interleaved with band DMAs)
            eng[X1_ENG].dma_start(
                x1t[:, g * X1B:(g + 1) * X1B],
                x1d.ap()[:, g * X1B:(g + 1) * X1B])

            stt = stpool.tile([128, ROWSZ], f16, tag="st")
            stv = stt[:].rearrange("p (r b u) -> p r b u", r=R, b=NWB)
            nb = PS_BANKS
            for q in range(NWB // nb):
                pst = pspool.tile([128, nb, 512], f32, tag="ps")
                for k in range(nb):
                    wb = q * nb + k
                    c0 = min(max(4 * wb - 4, 0), W - UB)
                    lhsT = x1t[:, g * X1B + wb * 128:g * X1B + (wb + 1) * 128]
                    rhs = x2v[:, 24 * g:24 * g + rg, c0:c0 + UB]
                    nc.tensor.matmul(pst[:, k, 0:nmm_g], lhsT, rhs,
                                     start=True, stop=True)

                has_lo = q == 0                       # k=0 is wb 0
                has_hi = q == NWB // nb - 1           # last k is wb 47
                ch = EVAC_PATTERN[ev % len(EVAC_PATTERN)]
                cp = {"v": nc.vector.tensor_copy, "s": nc.scalar.copy}[ch]
                ev += 1
                b0 = q * nb
                if has_lo:
                    # wb=0 clamped: psum u' = w2, dst u = u'+4
                    p0 = pst[:, 0, 0:nmm_g].rearrange("p (r u) -> p r u", r=rg)
                    cp(stv[:, 0:rg, 0, 4:12], p0[:, :, 0:8].copy())
                    if nb > 1:
                        src = pst[:, 1:nb, 0:nmm_g].rearrange(
                            "p a (r u) -> p r a u", r=rg).copy()
                        cp(stv[:, 0:rg, 1:nb, :], src)
                elif has_hi:
                    # wb=47 clamped: psum u' = w2-180, dst u = u'-4
                    p3 = pst[:, nb - 1, 0:nmm_g].rearrange(
                        "p (r u) -> p r u", r=rg)
                    cp(stv[:, 0:rg, NWB - 1, 0:8], p3[:, :, 4:12].copy())
                    if nb > 1:
                        src = pst[:, 0:nb - 1, 0:nmm_g].rearrange(
                            "p a (r u) -> p r a u", r=rg).copy()
                        cp(stv[:, 0:rg, b0:b0 + nb - 1, :], src)
                else:
                    src = pst[:, :, 0:nmm_g].rearrange(
                        "p a (r u) -> p r a u", r=rg).copy()
                    cp(stv[:, 0:rg, b0:b0 + nb, :], src)

            # band DMAs: one per rr-quad, clean partition steps
            for q in range(rg // 4):
                rr0 = 4 * q
                src = bass.AP(stt[:].tensor,
                              (4 * rr0) * ROWSZ + rr0 * (NWB * UB),
                              [[ROWSZ, QWIN], [1, QFREE]])
                dsto = (g * NQ + q) * QWIN * QFREE
                dst = bass.AP(outd.ap().tensor, dsto,
                              [[QFREE, QWIN], [1, QFREE]])
                be = BAND_ENG[bq % len(BAND_ENG)]
                bq += 1
                eng[be].dma_start(dst, src)

        if LOOP_N > 1:
            with tc.For_i(0, LOOP_N):
                body()
        else:
            body()

    nc.compile()
    return nc


def _get_nc():
    key = (EVAC_PATTERN, X2_ENG, X1_ENG, BAND_ENG, PS_BANKS, MS_ENG, ST_BUFS,
           LOOP_N)
    if key not in _CACHE:
        _CACHE[key] = _build()
    return _CACHE[key]


def _prep_inputs(x1, x2):
    x1 = np.asarray(x1, np.float32)
    x2 = np.asarray(x2, np.float32)
    in_maps = []
    for b in range(B):
        x1p = np.zeros((128, X1ROWS, W), np.float16)
        x1p[:, 8:8 + H, :] = x1[b]
        s0, s1, s2 = x1p.strides
        win = np.lib.stride_tricks.as_strided(
            x1p, shape=(128, NG, 32, NWB, WW),
            strides=(s0, 24 * s1, s1, WW * s2, s2))
        x1b = win.transpose(0, 1, 3, 2, 4).reshape(128, NG * X1B)
        in_maps.append({
            "x1h": np.ascontiguousarray(x1b),
            "x2h": x2[b].reshape(128, H * W).astype(np.float16)})
    return in_maps


def _decode(band, out81):
    """band: per-core [BANDSZ] f16/f32 -> out81 [81, H, W] f32 (unscaled)."""
    # (g*q, w//4=tprime, ww, j, wb, u)
    arr = band.reshape(NG * NQ, 12, WW, 4, NWB, UB)
    tm = np.empty((9, 9, X2ROWS, W), np.float32)  # (Delta, dj, rp, w')
    for ww in range(WW):
        for j in range(4):
            sub = arr[:, j:j + 9, ww, j, :, ww:ww + 9]  # (gq, Delta, wb, dj)
            tm[:, :, j::4, ww::WW] = sub.transpose(1, 3, 0, 2)
    for t in range(9):
        di_idx = 8 - t                            # di = 4 - t
        r2lo = di_idx                             # rp = h + di + 4
        out81[di_idx * 9:di_idx * 9 + 9] = tm[t, :, r2lo:r2lo + H, :]
    # w-boundary: reference zero-pads x2, so outputs with w+dj out of
    # bounds are exactly 0 (the device leaves garbage there).
    v = out81.reshape(9, 9, H, W)
    for dj in range(-MD, MD + 1):
        if dj < 0:
            v[:, dj + 4, :, 0:-dj] = 0
        elif dj > 0:
            v[:, dj + 4, :, W - dj:] = 0
    return out81


def kernel(x1, x2):
    from concourse.bass_utils import run_bass_kernel_spmd

    nc = _get_nc()
    in_maps = _prep_inputs(x1, x2)
    res = run_bass_kernel_spmd(nc, in_maps, core_ids=list(range(8)))

    inv_sqrt_c = np.float32(1.0 / math.sqrt(C))
    out = np.empty((B, NDISP - 1, H, W), np.float32)
    out81 = np.empty((NDISP, H, W), np.float32)
    for b in range(B):
        _decode(res.results[b]["band"].astype(np.float32), out81)
        out[b] = np.delete(out81, 40, axis=0) * inv_sqrt_c
    return out
